# revision 10
# baseline (speedup 1.0000x reference)
"""ChebGNN (K=3, 2 layers + mean-pool head) on 8 Trainium2 NeuronCores.

Sharding: nodes are partitioned across 8 cores (graph partition parallelism);
edges are partitioned by destination node.  Sparse propagation t -> L_hat @ t
is computed per 128-destination-node tile as a PSUM accumulation over 128-edge
tiles of  onehot(col%128)^T @ gathered_src_rows,  where the one-hot selection
matrix is built on-chip (iota + is_equal) and source rows are fetched with
dma_gather from the pre-scaled (D^-1/2) full feature table.  The full table is
rebuilt between propagations with an AllGather.  dma_gather takes int16
indices, so the node table is split into a lo half (< 32768) and a hi half;
edges are bucketed per (dst tile, half).  Gather payloads and one-hot matrices
are bf16 (exact 0/1), accumulation stays fp32 in PSUM.
"""

import numpy as np
import ml_dtypes

from concourse import bass, mybir, tile, bacc
from concourse import bass_utils

P = 128
FP = mybir.dt.float32
BF = mybir.dt.bfloat16
I16 = mybir.dt.int16
I32 = mybir.dt.int32

N_NODES = 50000
C = 128
N_CORES = 8
NPAD = 50176  # 392 * 128
SHARD = NPAD // N_CORES  # 6272
NT = SHARD // P  # 49 node tiles per core
LO = 32768  # int16 index boundary for dma_gather
G_CHUNK = 1  # dst tiles per gather call

_AF = mybir.ActivationFunctionType
_OP = mybir.AluOpType
BF_NP = ml_dtypes.bfloat16


def _wrap16(flat):
    """dma_gather index layout: idx i at [i%16 (replicated x8), i//16]."""
    w16 = flat.reshape(-1, 16).T  # [16, n//16]
    return np.ascontiguousarray(np.tile(w16, (8, 1)))  # [128, n//16]


def _bucket(keys_sorted, vals, nt_total):
    """Per 128-wide key bucket: t_max and slot arrays."""
    starts = np.searchsorted(keys_sorted, np.arange(0, nt_total * P + 1, P))
    counts = starts[1:] - starts[:-1]
    t_max = max(1, int(np.ceil(counts.max() / P)))
    tile_of = (keys_sorted // P).astype(np.int64)
    rank = np.arange(len(keys_sorted), dtype=np.int64) - starts[tile_of]
    return t_max, tile_of, rank


def prep_inputs(x, edge_index, W1, b1, W2, b2, Wlin, blin):
    row = np.asarray(edge_index[0], np.int64)
    col = np.asarray(edge_index[1], np.int64)

    # ---- destination-bucketed edges, split by source half ----
    halves = []
    t_maxes = []
    for h in range(2):
        m = (row < LO) if h == 0 else (row >= LO)
        r_h, c_h = row[m], col[m]
        order = np.argsort(c_h, kind="stable")
        cs, rs = c_h[order], r_h[order]
        t_max, tile_of, rank = _bucket(cs, None, NPAD // P)
        core = tile_of // NT
        ltile = tile_of % NT
        W = NT * t_max
        gi = np.zeros((N_CORES, W * P), np.int16)
        cm = np.full((N_CORES, W, P), 255.0, np.float32)
        kk = ltile * t_max + rank // P
        pp = rank % P
        gi[core, kk * P + pp] = (rs - h * LO).astype(np.int16)
        cm[core, kk, pp] = (cs % P).astype(np.float32)
        gi16 = np.stack([_wrap16(gi[c]) for c in range(N_CORES)])
        cmT = np.ascontiguousarray(
            cm.transpose(0, 2, 1))  # [cores,128,W] fp32
        halves.append((gi16, cmT))
        t_maxes.append(t_max)
    (gl16, cml), (gh16, cmh) = halves
    t_lo, t_hi = t_maxes

    # ---- source-bucketed edges for degree ----
    rsrt = np.sort(row)
    tr_max, tile_of, rank = _bucket(rsrt, None, NPAD // P)
    core = tile_of // NT
    ltile = tile_of % NT
    Wr = NT * tr_max
    rm = np.full((N_CORES, Wr, P), 255.0, np.float32)
    rm[core, ltile * tr_max + rank // P, rank % P] = (rsrt % P)
    rmT = np.ascontiguousarray(rm.transpose(0, 2, 1))

    xp = np.zeros((NPAD, C), np.float32)
    xp[:N_NODES] = np.asarray(x, np.float32)

    node_id = (np.arange(N_CORES)[:, None, None] * SHARD
               + np.arange(NT)[None, None, :] * P
               + np.arange(P)[None, :, None])
    mask = (node_id < N_NODES).astype(np.float32)

    in_maps = []
    for c in range(N_CORES):
        in_maps.append(dict(
            xsh=np.ascontiguousarray(xp[c * SHARD:(c + 1) * SHARD]),
            gidx_lo=gl16[c], gidx_hi=gh16[c],
            cmod_lo=cml[c], cmod_hi=cmh[c],
            rmod=rmT[c],
            mask=np.ascontiguousarray(mask[c]),
            iotac=np.tile(np.arange(P, dtype=np.float32), (P, 1)),
            W1=np.asarray(W1, np.float32), b1=np.asarray(b1, np.float32),
            W2=np.asarray(W2, np.float32), b2=np.asarray(b2, np.float32),
            Wlin=np.asarray(Wlin, np.float32),
            blin=np.asarray(blin, np.float32),
        ))
    return in_maps, t_lo, t_hi, tr_max


def build_program(t_lo, t_hi, tr_max):
    nc = bacc.Bacc("TRN2", target_bir_lowering=False, debug=False,
                   enable_asserts=False, num_devices=N_CORES)
    Wl_, Wh_, Wr_ = NT * t_lo, NT * t_hi, NT * tr_max

    xsh = nc.dram_tensor("xsh", [SHARD, C], FP, kind="ExternalInput").ap()
    gidx_lo = nc.dram_tensor("gidx_lo", [P, Wl_ * 8], I16,
                             kind="ExternalInput").ap()
    gidx_hi = nc.dram_tensor("gidx_hi", [P, Wh_ * 8], I16,
                             kind="ExternalInput").ap()
    cmod_lo = nc.dram_tensor("cmod_lo", [P, Wl_], FP,
                             kind="ExternalInput").ap()
    cmod_hi = nc.dram_tensor("cmod_hi", [P, Wh_], FP,
                             kind="ExternalInput").ap()
    rmod = nc.dram_tensor("rmod", [P, Wr_], FP, kind="ExternalInput").ap()
    maskd = nc.dram_tensor("mask", [P, NT], FP, kind="ExternalInput").ap()
    iotad = nc.dram_tensor("iotac", [P, P], FP, kind="ExternalInput").ap()
    W1d = nc.dram_tensor("W1", [3, C, C], FP, kind="ExternalInput").ap()
    b1d = nc.dram_tensor("b1", [C], FP, kind="ExternalInput").ap()
    W2d = nc.dram_tensor("W2", [3, C, C], FP, kind="ExternalInput").ap()
    b2d = nc.dram_tensor("b2", [C], FP, kind="ExternalInput").ap()
    Wld = nc.dram_tensor("Wlin", [C, 2], FP, kind="ExternalInput").ap()
    bld = nc.dram_tensor("blin", [2], FP, kind="ExternalInput").ap()
    h_out = nc.dram_tensor("h_out", [SHARD, C], FP, kind="ExternalOutput").ap()
    out_fin = nc.dram_tensor("out_final", [1, 2], FP,
                             kind="ExternalOutput").ap()

    def idram(name, shape, dt=BF):
        return nc.dram_tensor(name, shape, dt, kind="Internal").ap()

    xp_sh = idram("xp_sh", [SHARD, C])
    xp_full = idram("xp_full", [NPAD, C])
    t1p_sh = idram("t1p_sh", [SHARD, C])
    t1p_full = idram("t1p_full", [NPAD, C])
    h1p_sh = idram("h1p_sh", [SHARD, C])
    h1p_full = idram("h1p_full", [NPAD, C])
    u1p_sh = idram("u1p_sh", [SHARD, C])
    u1p_full = idram("u1p_full", [NPAD, C])
    cs_in = idram("cs_in", [P, 1], FP)
    cs_out = idram("cs_out", [P, 1], FP)

    rg = [list(range(N_CORES))]
    from concourse.masks import make_identity

    with tile.TileContext(nc) as tc:
        with (
            tc.tile_pool(name="const", bufs=1) as cp,
            tc.tile_pool(name="res", bufs=1) as rp,
            tc.tile_pool(name="vlo", bufs=2) as vlp,
            tc.tile_pool(name="vhi", bufs=2) as vhp,
            tc.tile_pool(name="m0", bufs=4) as mp,
            tc.tile_pool(name="work", bufs=4) as wp,
            tc.tile_pool(name="trp", bufs=6) as trp,
            tc.tile_pool(name="pp", bufs=2, space="PSUM") as pp,
            tc.tile_pool(name="tp", bufs=2, space="PSUM") as tp,
            tc.tile_pool(name="hp", bufs=2, space="PSUM") as hp,
            tc.tile_pool(name="sp", bufs=1, space="PSUM") as sp,
        ):
            # ---------- constants ----------
            iota_f32 = cp.tile([P, P], FP)
            nc.sync.dma_start(iota_f32[:], iotad[:])
            iota_b = cp.tile([P, P], BF)
            nc.vector.tensor_copy(iota_b[:], iota_f32[:])
            ident = cp.tile([P, P], FP)
            make_identity(nc, ident[:])
            ones_b = cp.tile([P, 1], BF)
            nc.vector.memset(ones_b[:], 1.0)
            ones_f = cp.tile([P, 1], FP)
            nc.vector.memset(ones_f[:], 1.0)

            W1_sb = cp.tile([P, 3 * C], FP)
            W2_sb = cp.tile([P, 3 * C], FP)
            for k in range(3):
                nc.sync.dma_start(W1_sb[:, k * C:(k + 1) * C], W1d[k])
                nc.sync.dma_start(W2_sb[:, k * C:(k + 1) * C], W2d[k])
            Wl_sb = cp.tile([P, 2], FP)
            nc.sync.dma_start(Wl_sb[:], Wld[:])
            b1c = cp.tile([P, 1], FP)
            nc.sync.dma_start(b1c[:], b1d[:, None])
            b2c = cp.tile([P, 1], FP)
            nc.sync.dma_start(b2c[:], b2d[:, None])
            blc = cp.tile([1, 2], FP)
            nc.sync.dma_start(blc[:], bld[None, :])
            mask_sb = cp.tile([P, NT], FP)
            nc.sync.dma_start(mask_sb[:], maskd[:])

            gl_sb = cp.tile([P, Wl_ * 8], I16)
            nc.sync.dma_start(gl_sb[:], gidx_lo[:])
            gh_sb = cp.tile([P, Wh_ * 8], I16)
            nc.sync.dma_start(gh_sb[:], gidx_hi[:])
            cl_sb = cp.tile([P, Wl_], FP)
            nc.sync.dma_start(cl_sb[:], cmod_lo[:])
            ch_sb = cp.tile([P, Wh_], FP)
            nc.sync.dma_start(ch_sb[:], cmod_hi[:])
            rm_sb = cp.tile([P, Wr_], FP)
            nc.sync.dma_start(rm_sb[:], rmod[:])

            # ---------- degree (column form) ----------
            deg = cp.tile([P, NT], FP)
            for t in range(NT):
                ps_deg = pp.tile([P, 1], FP, tag="ppt")
                for j in range(tr_max):
                    m0r = mp.tile([P, P], BF)
                    nc.any.tensor_scalar(
                        m0r[:], iota_b[:],
                        scalar1=rm_sb[:, t * tr_max + j:t * tr_max + j + 1],
                        scalar2=None, op0=_OP.is_equal)
                    nc.tensor.matmul(ps_deg[:], lhsT=m0r[:], rhs=ones_b[:],
                                     start=(j == 0), stop=(j == tr_max - 1))
                nc.vector.tensor_copy(deg[:, t:t + 1], ps_deg[:])

            zfl = cp.tile([P, NT], FP)
            nc.vector.tensor_scalar(zfl[:], deg[:], scalar1=0.0, scalar2=None,
                                    op0=_OP.is_equal)
            safe = cp.tile([P, NT], FP)
            nc.vector.tensor_tensor(safe[:], deg[:], zfl[:], op=_OP.add)
            sq_ = cp.tile([P, NT], FP)
            nc.scalar.activation(sq_[:], safe[:], _AF.Sqrt)
            rs_ = cp.tile([P, NT], FP)
            nc.vector.reciprocal(rs_[:], sq_[:])
            nzm = cp.tile([P, NT], FP)
            nc.vector.tensor_scalar(nzm[:], zfl[:], scalar1=-1.0, scalar2=1.0,
                                    op0=_OP.mult, op1=_OP.add)
            dinv = cp.tile([P, NT], FP)
            nc.vector.tensor_tensor(dinv[:], rs_[:], nzm[:], op=_OP.mult)
            mdinv = cp.tile([P, NT], FP)
            nc.vector.tensor_scalar(mdinv[:], dinv[:], scalar1=-1.0,
                                    scalar2=None, op0=_OP.mult)
            m2dinv = cp.tile([P, NT], FP)
            nc.vector.tensor_scalar(m2dinv[:], dinv[:], scalar1=-2.0,
                                    scalar2=None, op0=_OP.mult)
            mdinv2 = cp.tile([P, NT], FP)
            nc.vector.tensor_tensor(mdinv2[:], mdinv[:], dinv[:], op=_OP.mult)

            # ---------- x shard, x' = dinv*x ----------
            x_res = rp.tile([P, SHARD], FP)
            for d in range(NT):
                nc.sync.dma_start(x_res[:, d * P:(d + 1) * P],
                                  xsh[d * P:(d + 1) * P, :])
                xpt = wp.tile([P, P], BF, tag="agout")
                nc.vector.tensor_scalar(xpt[:], x_res[:, d * P:(d + 1) * P],
                                        scalar1=dinv[:, d:d + 1], scalar2=None,
                                        op0=_OP.mult)
                nc.sync.dma_start(xp_sh[d * P:(d + 1) * P, :], xpt[:])

            nc.gpsimd.collective_compute(
                "AllGather", _OP.bypass, replica_groups=rg,
                ins=[xp_sh[:]], outs=[xp_full[:]])

            t1_res = rp.tile([P, SHARD], FP)
            h1_res = rp.tile([P, SHARD], FP)

            # ---------- propagation machinery ----------
            def prop(src_full, epilogue):
                GW = 8  # max edge-tile columns (1024 idxs) per dma_gather
                for c0 in range(0, NT, G_CHUNK):
                    g = min(G_CHUNK, NT - c0)
                    vlo = vlp.tile([P, G_CHUNK * t_lo * P], BF, tag="vlo")
                    vhi = vhp.tile([P, G_CHUNK * t_hi * P], BF, tag="vhi")
                    for vt, gsb, tm, srcv in (
                            (vlo, gl_sb, t_lo, src_full[:LO, :]),
                            (vhi, gh_sb, t_hi, src_full[LO:, :])):
                        ntile = g * tm
                        for a in range(0, ntile, GW):
                            w = min(GW, ntile - a)
                            nc.gpsimd.dma_gather(
                                vt[:, a * P:(a + w) * P].rearrange(
                                    "p (t e) -> p t e", e=P),
                                srcv,
                                gsb[:, (c0 * tm + a) * 8:
                                    (c0 * tm + a + w) * 8],
                                w * P, w * P, C)
                    for dl in range(g):
                        d = c0 + dl
                        ps = pp.tile([P, P], FP, tag="ppt")
                        nmm = t_lo + t_hi
                        i = 0
                        for src_v, csb, tm in ((vlo, cl_sb, t_lo),
                                               (vhi, ch_sb, t_hi)):
                            for j in range(tm):
                                kc = d * tm + j
                                m0 = mp.tile([P, P], BF)
                                nc.any.tensor_scalar(
                                    m0[:], iota_b[:],
                                    scalar1=csb[:, kc:kc + 1],
                                    scalar2=None, op0=_OP.is_equal)
                                vs = src_v[:, (dl * tm + j) * P:
                                           (dl * tm + j + 1) * P]
                                nc.tensor.matmul(ps[:], lhsT=m0[:], rhs=vs,
                                                 start=(i == 0),
                                                 stop=(i == nmm - 1))
                                i += 1
                        epilogue(d, ps)

            def transpose_to(src_ap):
                pst = tp.tile([P, P], FP, tag="tpt")
                nc.tensor.transpose(pst[:], src_ap, ident[:])
                out = trp.tile([P, P], FP, tag="tr")
                nc.vector.tensor_copy(out[:], pst[:])
                return out

            # ---------- layer 1 prop 1 ----------
            def epi_t1(d, ps):
                sl = slice(d * P, (d + 1) * P)
                nc.vector.tensor_scalar(t1_res[:, sl], ps[:],
                                        scalar1=mdinv[:, d:d + 1],
                                        scalar2=None, op0=_OP.mult)
                t1p = wp.tile([P, P], BF, tag="agout")
                nc.vector.tensor_scalar(t1p[:], ps[:],
                                        scalar1=mdinv2[:, d:d + 1],
                                        scalar2=None, op0=_OP.mult)
                nc.sync.dma_start(t1p_sh[sl, :], t1p[:])

            prop(xp_full, epi_t1)
            nc.gpsimd.collective_compute(
                "AllGather", _OP.bypass, replica_groups=rg,
                ins=[t1p_sh[:]], outs=[t1p_full[:]])

            # ---------- dense (Chebyshev combine) ----------
            cs_ps = sp.tile([P, 1], FP, tag="cs")

            def dense_layer(d, t0_ap, t1_ap, t2_ap, Wsb, bcol, dst_res,
                            dst_prime, final=False):
                sl = slice(d * P, (d + 1) * P)
                tT = [transpose_to(a) for a in (t0_ap, t1_ap, t2_ap)]
                psh = hp.tile([P, P], FP, tag="hpt")
                for k in range(3):
                    nc.tensor.matmul(psh[:], lhsT=Wsb[:, k * C:(k + 1) * C],
                                     rhs=tT[k][:], start=(k == 0),
                                     stop=(k == 2))
                hT = wp.tile([P, P], FP, tag="hT")
                nc.vector.tensor_scalar(hT[:], psh[:], scalar1=bcol[:],
                                        scalar2=0.0, op0=_OP.add, op1=_OP.max)
                hps = tp.tile([P, P], FP, tag="tpt")
                nc.tensor.transpose(hps[:], hT[:], ident[:])
                if final:
                    hm = wp.tile([P, P], FP, tag="hm")
                    nc.vector.tensor_scalar(hm[:], hps[:],
                                            scalar1=mask_sb[:, d:d + 1],
                                            scalar2=None, op0=_OP.mult)
                    nc.sync.dma_start(h_out[sl, :], hm[:])
                    nc.tensor.matmul(cs_ps[:], lhsT=hm[:], rhs=ones_f[:],
                                     start=(d == 0), stop=(d == NT - 1))
                else:
                    nc.vector.tensor_copy(dst_res[:, sl], hps[:])
                    hp_t = wp.tile([P, P], BF, tag="agout")
                    nc.vector.tensor_scalar(hp_t[:], hps[:],
                                            scalar1=dinv[:, d:d + 1],
                                            scalar2=None, op0=_OP.mult)
                    nc.sync.dma_start(dst_prime[sl, :], hp_t[:])

            def epi_t2_dense(d, ps):
                sl = slice(d * P, (d + 1) * P)
                t2 = wp.tile([P, P], FP, tag="t2")
                nc.vector.tensor_scalar(t2[:], ps[:],
                                        scalar1=m2dinv[:, d:d + 1],
                                        scalar2=None, op0=_OP.mult)
                nc.vector.tensor_tensor(t2[:], t2[:], x_res[:, sl],
                                        op=_OP.subtract)
                dense_layer(d, x_res[:, sl], t1_res[:, sl], t2[:],
                            W1_sb, b1c, h1_res, h1p_sh)

            prop(t1p_full, epi_t2_dense)
            nc.gpsimd.collective_compute(
                "AllGather", _OP.bypass, replica_groups=rg,
                ins=[h1p_sh[:]], outs=[h1p_full[:]])

            # ---------- layer 2 prop 1 (reuse t1_res) ----------
            def epi_u1(d, ps):
                sl = slice(d * P, (d + 1) * P)
                nc.vector.tensor_scalar(t1_res[:, sl], ps[:],
                                        scalar1=mdinv[:, d:d + 1],
                                        scalar2=None, op0=_OP.mult)
                u1p = wp.tile([P, P], BF, tag="agout")
                nc.vector.tensor_scalar(u1p[:], ps[:],
                                        scalar1=mdinv2[:, d:d + 1],
                                        scalar2=None, op0=_OP.mult)
                nc.sync.dma_start(u1p_sh[sl, :], u1p[:])

            prop(h1p_full, epi_u1)
            nc.gpsimd.collective_compute(
                "AllGather", _OP.bypass, replica_groups=rg,
                ins=[u1p_sh[:]], outs=[u1p_full[:]])

            # ---------- layer 2 prop 2 + dense + colsum ----------
            def epi_u2_dense(d, ps):
                sl = slice(d * P, (d + 1) * P)
                u2 = wp.tile([P, P], FP, tag="t2")
                nc.vector.tensor_scalar(u2[:], ps[:],
                                        scalar1=m2dinv[:, d:d + 1],
                                        scalar2=None, op0=_OP.mult)
                nc.vector.tensor_tensor(u2[:], u2[:], h1_res[:, sl],
                                        op=_OP.subtract)
                dense_layer(d, h1_res[:, sl], t1_res[:, sl], u2[:],
                            W2_sb, b2c, None, None, final=True)

            prop(u1p_full, epi_u2_dense)

            # ---------- mean-pool head ----------
            cs_sb = wp.tile([P, 1], FP, tag="cs1")
            nc.vector.tensor_copy(cs_sb[:], cs_ps[:])
            nc.sync.dma_start(cs_in[:], cs_sb[:])
            nc.gpsimd.collective_compute(
                "AllReduce", _OP.add, replica_groups=rg,
                ins=[cs_in[:]], outs=[cs_out[:]])
            mean_c = wp.tile([P, 1], FP, tag="cs2")
            nc.sync.dma_start(mean_c[:], cs_out[:])
            nc.vector.tensor_scalar(mean_c[:], mean_c[:],
                                    scalar1=1.0 / N_NODES, scalar2=None,
                                    op0=_OP.mult)
            fin_ps = sp.tile([1, 2], FP, tag="fin")
            nc.tensor.matmul(fin_ps[:], lhsT=mean_c[:], rhs=Wl_sb[:],
                             start=True, stop=True)
            fin_sb = wp.tile([1, 2], FP, tag="fin1")
            nc.vector.tensor_tensor(fin_sb[:], fin_ps[:], blc[:], op=_OP.add)
            nc.sync.dma_start(out_fin[:], fin_sb[:])

    nc.compile()
    return nc


_PROGRAM_CACHE = {}


def _get_program(t_lo, t_hi, tr_max):
    key = (t_lo, t_hi, tr_max)
    if key not in _PROGRAM_CACHE:
        _PROGRAM_CACHE[key] = build_program(t_lo, t_hi, tr_max)
    return _PROGRAM_CACHE[key]


def kernel(x, edge_index, W1, b1, W2, b2, Wlin, blin):
    in_maps, t_lo, t_hi, tr_max = prep_inputs(x, edge_index, W1, b1, W2, b2,
                                              Wlin, blin)
    nc = _get_program(t_lo, t_hi, tr_max)
    res = bass_utils.run_bass_kernel_spmd(
        nc, in_maps, core_ids=list(range(N_CORES)), trace=False)
    h = np.concatenate([res.results[c]["h_out"] for c in range(N_CORES)],
                       axis=0)[:N_NODES]
    out = res.results[0]["out_final"]
    return (np.asarray(out, np.float32), np.ascontiguousarray(h, np.float32))
